# revision 17
# baseline (speedup 1.0000x reference)
"""GTN Bass kernel v3 for 8 Trainium2 NeuronCores.

All three A-contractions run in the "A-stationary" orientation: the host
pre-combines the five edge-type adjacencies into per-channel, per-pass
matrices B[t,c] = sum_e f_t[c,e] A_e (softmax coefficients folded in), and
each pass computes X_out[n,w] = sum_m B[m,n] X_in[m,w] as 128x128-stationary
matmuls with the skinny [128,64] operand moving.  Per pass that is
C*MB*J = 64 matmuls of 64 moving columns (4096 PE column-cycles) versus the
v2 design's 80 matmuls of 256 columns (20480) - and no per-(c,e) DVE
prescales, because the coefficients live in B.

Pass outputs land node-major in PSUM ([node,64] per channel/block), which is
exactly the row layout the inter-pass AllGather wants, so the only
transposes left are the two [128,128] flips in front of the MLP tail (which
contracts over features).

Normalization vectors are the same exact host-side precomputes as v2:
  r1[c]  = 1/colsum(H1_c)   (pass-2 output row scaling)
  d2inv  = 1/(N*colsum(Q_c)) (pass-3 output scaling, GCN degree norm folded)
applied as per-partition scalars in the node-major layout.

Sharding: core k owns columns [256k, 256k+256) of every B[t,c] (host-sliced,
cast to bf16); AllGathers rebuild the full skinny operand between passes.
"""

import sys

import numpy as np

sys.path.insert(0, "/opt/trn_rl_repo")

import ml_dtypes

import concourse.bass as bass
from concourse import bacc
import concourse.mybir as mybir
from concourse.bass import ds
from concourse.bass_utils import run_bass_kernel_spmd
from concourse.masks import make_identity
from concourse.tile import TileContext

E, C, N = 5, 2, 2048
W_IN, W_OUT, NUM_CLASS = 256, 64, 8
NCORES = 8
S = N // NCORES          # 256 shard columns per core
P = 128
J = N // P               # 16 contraction chunks
MB = S // P              # 2 output row blocks per shard
W2 = C * W_OUT           # 128: stacked channel width
T = 3                    # passes

F32 = mybir.dt.float32
BF16 = mybir.dt.bfloat16
ADD = mybir.AluOpType.add
MULT = mybir.AluOpType.mult
MAX = mybir.AluOpType.max
COPY = mybir.ActivationFunctionType.Copy


def _softmax(w):
    e = np.exp(w - w.max(axis=1, keepdims=True))
    return e / e.sum(axis=1, keepdims=True)


def _build(reps=1, nocc=False, stages=3, skipcc=False, warm=0):
    """Per-core SPMD program.  All softmax coefficients are folded into the
    host-combined B matrices; the program itself has no data-dependent
    immediates.  stages<3 truncates the per-rep body (timing experiments).
    skipcc replaces each AllGather with one local 64KB DMA (timing lower
    bound).  warm>0 emits that many dependency-free 64-col PE matmuls into
    each collective window so the PE p-state never drops."""
    nc = bacc.Bacc(None, target_bir_lowering=False)

    b_in = nc.declare_dram_parameter("b_sh", [T, C, N, S], BF16, isOutput=False)
    g_in = nc.declare_dram_parameter("g", [N, W_OUT], BF16, isOutput=False)
    r1_in = nc.declare_dram_parameter("r1p", [P, MB, C], F32, isOutput=False)
    d2_in = nc.declare_dram_parameter("d2p", [P, MB, C], F32, isOutput=False)
    l1_in = nc.declare_dram_parameter("lin1w", [W2, W_OUT], BF16, isOutput=False)
    b1_in = nc.declare_dram_parameter("lin1b", [W_OUT, 1], F32, isOutput=False)
    l2_in = nc.declare_dram_parameter("lin2w", [W_OUT, NUM_CLASS], BF16, isOutput=False)
    y_out = nc.declare_dram_parameter("y_t", [NUM_CLASS, S], F32, isOutput=True)

    ag1_in = nc.dram_tensor("ag1_in", [S, W2], BF16)
    ag1_out = nc.dram_tensor("ag1_out", [N, W2], BF16, addr_space="Shared")
    ag2_in = nc.dram_tensor("ag2_in", [S, W2], BF16)
    ag2_out = nc.dram_tensor("ag2_out", [N, W2], BF16, addr_space="Shared")
    groups = [list(range(NCORES))]

    with TileContext(nc) as tc:
        with (
            tc.tile_pool(name="bbuf", bufs=1) as b_pool,
            tc.tile_pool(name="wbuf", bufs=1) as w_pool,
            tc.tile_pool(name="mv", bufs=2) as mv_pool,
            tc.tile_pool(name="work", bufs=2) as wk,
            tc.tile_pool(name="psmain", bufs=1, space="PSUM") as pp,
            tc.tile_pool(name="pstr", bufs=1, space="PSUM") as pt,
            tc.tile_pool(name="pstail", bufs=1, space="PSUM") as pz,
        ):
            # ---- persistent SBUF loads -------------------------------------
            # row r of every row-indexed [N, *] operand maps to
            # (partition, chunk) = (r // J, r % J): per-partition DRAM reads
            # are fully contiguous.
            b_t = b_pool.tile([P, T, C, J, S], BF16, tag="B")
            nc.sync.dma_start(out=b_t[:, :, :, :, :],
                              in_=b_in[:].rearrange("t c (p j) m -> p t c j m",
                                                    p=P))
            g_t = w_pool.tile([P, J, W_OUT], BF16, tag="g")
            nc.sync.dma_start(out=g_t[:, :, :],
                              in_=g_in[:].rearrange("(p j) m -> p j m", p=P))
            r1_t = w_pool.tile([P, MB, C], F32, tag="r1")
            nc.sync.dma_start(out=r1_t[:, :, :], in_=r1_in[:])
            d2_t = w_pool.tile([P, MB, C], F32, tag="d2")
            nc.sync.dma_start(out=d2_t[:, :, :], in_=d2_in[:])
            l1_t = w_pool.tile([W2, W_OUT], BF16, tag="l1")
            nc.sync.dma_start(out=l1_t[:, :], in_=l1_in[:])
            b1_t = w_pool.tile([W_OUT, 1], F32, tag="b1")
            nc.sync.dma_start(out=b1_t[:, :], in_=b1_in[:])
            l2_t = w_pool.tile([W_OUT, NUM_CLASS], BF16, tag="l2")
            nc.sync.dma_start(out=l2_t[:, :], in_=l2_in[:])
            ident = w_pool.tile([P, P], BF16, tag="ident")
            make_identity(nc, ident[:, :])

            def run_pass(t, rhs_of, name, sink):
                """4 accumulation chains, nb-major, each [128,64] node-major.
                rhs_of(j, c) supplies the moving operand for chunk j; sink(c,
                nb, ps) consumes each chain's PSUM as it completes so the
                first half of the output can stage while the second half is
                still accumulating."""
                for nb in range(MB):
                    for c in range(C):
                        p_ = pp.tile([P, W_OUT], F32, tag=f"ps{c}{nb}",
                                     name=f"ps_{name}_{c}{nb}")
                        for j in range(J):
                            nc.tensor.matmul(
                                out=p_[:, :],
                                lhsT=b_t[:, t, c, j, ds(nb * P, P)],
                                rhs=rhs_of(j, c),
                                start=(j == 0),
                                stop=(j == J - 1),
                            )
                        sink(c, nb, p_)

            def run_warm(n, name):
                if n <= 0:
                    return
                psw = pp.tile([P, W_OUT], F32, tag="warm", name=f"warm_{name}")
                for i in range(n):
                    nc.tensor.matmul(out=psw[:, :],
                                     lhsT=b_t[:, 0, 0, 0, ds(0, P)],
                                     rhs=b_t[:, 0, 0, 1, ds(0, W_OUT)],
                                     start=True, stop=True)

            H = J // 2

            def all_gather(ag_in, ag_out, osb, name):
                nc.sync.dma_start(out=ag_in[:].rearrange("(m p) w -> p m w", p=P),
                                  in_=osb[:, :, :])
                if skipcc:
                    nc.sync.dma_start(out=ag_out[ds(0, S), :], in_=ag_in[:])
                elif nocc:
                    for kk in range(NCORES):
                        nc.sync.dma_start(out=ag_out[ds(kk * S, S), :],
                                          in_=ag_in[:])
                else:
                    nc.gpsimd.collective_compute(
                        "AllGather", mybir.AluOpType.bypass,
                        replica_groups=groups,
                        ins=[ag_in[:]], outs=[ag_out[:]])
                run_warm(warm, name)
                mv = mv_pool.tile([P, J, W2], BF16, tag="mvin", name=f"mv_{name}")
                nc.sync.dma_start(out=mv[:, :, :],
                                  in_=ag_out[:].rearrange("(p j) m -> p j m", p=P))
                return lambda j, c: mv[:, j, ds(W_OUT * c, W_OUT)]

            prev_tail = [None]
            for _rep in range(reps):
                if _rep > 0 and prev_tail[0] is not None:
                    # serialize reps so the reps-slope measures latency
                    zt = wk.tile([NUM_CLASS, 1], F32, tag="zdep",
                                 name=f"zdep_{_rep}")
                    nc.vector.tensor_scalar(zt[:, :], prev_tail[0],
                                            0.0, None, MULT)
                    nc.vector.tensor_tensor(g_t[0:NUM_CLASS, 0, ds(0, 1)],
                                            g_t[0:NUM_CLASS, 0, ds(0, 1)],
                                            zt[:, :], ADD)

                # ---- pass 1: X1[c] = B[0,c]^T g ----------------------------
                osb1 = wk.tile([P, MB, W2], BF16, tag="osb1", name=f"osb1_{_rep}")
                run_pass(0, lambda j, c: g_t[:, j, :], f"p1_{_rep}",
                         lambda c, nb, p_: nc.vector.tensor_copy(
                             osb1[:, nb, ds(W_OUT * c, W_OUT)], p_[:, :]))
                if stages < 2:
                    prev_tail[0] = osb1[0:NUM_CLASS, 0, ds(0, 1)]
                    continue
                mv1 = all_gather(ag1_in, ag1_out, osb1, f"1_{_rep}")

                # ---- pass 2: X2[c] = r1[c] * (B[1,c]^T X1[c]) --------------
                osb2 = wk.tile([P, MB, W2], BF16, tag="osb2", name=f"osb2_{_rep}")
                run_pass(1, mv1, f"p2_{_rep}",
                         lambda c, nb, p_: nc.vector.tensor_scalar(
                             osb2[:, nb, ds(W_OUT * c, W_OUT)], p_[:, :],
                             r1_t[:, nb, ds(c, 1)], None, MULT))
                if stages < 3:
                    prev_tail[0] = osb2[0:NUM_CLASS, 0, ds(0, 1)]
                    continue
                mv2 = all_gather(ag2_in, ag2_out, osb2, f"2_{_rep}")

                # ---- pass 3 + GCN scale/relu + transpose + MLP tail --------
                xt = wk.tile([P, MB, W2], BF16, tag="xt", name=f"xt_{_rep}")
                run_pass(2, mv2, f"p3_{_rep}",
                         lambda c, nb, p_: nc.vector.tensor_scalar(
                             xt[:, nb, ds(W_OUT * c, W_OUT)], p_[:, :],
                             d2_t[:, nb, ds(c, 1)], 0.0, MULT, MAX))
                xb = wk.tile([P, S], BF16, tag="xb", name=f"xb_{_rep}")
                for nb in range(MB):
                    pst = pt.tile([P, P], BF16, tag="pst", name=f"t3_{_rep}_{nb}")
                    nc.tensor.transpose(pst[:, :], xt[:, nb, :], ident[:, :])
                    nc.vector.tensor_copy(xb[:, ds(nb * P, P)], pst[:, :])
                psz = pz.tile([W_OUT, S], F32, tag="psz", name=f"psz_{_rep}")
                nc.tensor.matmul(out=psz[:, :], lhsT=l1_t[:, :], rhs=xb[:, :],
                                 start=True, stop=True)
                z = wk.tile([W_OUT, S], BF16, tag="z", name=f"z_{_rep}")
                nc.vector.tensor_scalar(z[:, :], psz[:, :], b1_t[:, ds(0, 1)],
                                        0.0, ADD, MAX)
                psy = pz.tile([NUM_CLASS, S], F32, tag="psy", name=f"psy_{_rep}")
                nc.tensor.matmul(out=psy[:, :], lhsT=l2_t[:, :], rhs=z[:, :],
                                 start=True, stop=True)
                ysb = wk.tile([NUM_CLASS, S], F32, tag="ysb", name=f"ysb_{_rep}")
                nc.vector.tensor_copy(ysb[:, :], psy[:, :])
                nc.sync.dma_start(out=y_out[:, :], in_=ysb[:, :])
                run_warm(warm // 2, f"t_{_rep}")
                prev_tail[0] = ysb[:, ds(0, 1)]

    nc.finalize()
    return nc


def _host_prep(A, h, gt_w1a, gt_w1b, gt_w2, gcn_w, gcn_b, lin1_w, lin2_w):
    A = np.asarray(A, dtype=np.float32)
    f1a = _softmax(np.asarray(gt_w1a, dtype=np.float64))
    f1b = _softmax(np.asarray(gt_w1b, dtype=np.float64))
    f2 = _softmax(np.asarray(gt_w2, dtype=np.float64))

    g = (np.asarray(h, np.float32) @ np.asarray(gcn_w, np.float32)
         + np.asarray(gcn_b, np.float32))                       # [N, 64]

    # B[t,c] = sum_e f_t[c,e] A_e, per pass t in (f1a, f1b, f2)
    fs = np.stack([f1a, f1b, f2]).astype(np.float32)            # [T, C, E]
    B = np.einsum('tce,enm->tcnm', fs, A)                       # [T, C, N, N]

    Ad = A.astype(np.float64)
    sA = Ad.sum(axis=1)                                         # [E, N] colsums
    s1 = f1a @ sA                                               # [C, N]
    # u1[c] = colsum(H1_c) = sum_e f1b[c,e] * (A_e.T @ s1[c])
    u1 = np.stack([
        sum(f1b[c, e] * (Ad[e].T @ s1[c]) for e in range(E)) for c in range(C)
    ])                                                          # [C, N]
    r1 = np.where(u1 != 0, 1.0 / u1, 0.0)
    deg2 = f2 @ sA                                              # [C, N]
    d2inv = np.where(deg2 != 0, 1.0 / (N * deg2), 0.0)          # [C, N]

    def _perp(v, sl):
        # [C, S] slice -> [P, MB, C] per-partition scalars (node-major)
        return np.ascontiguousarray(
            v[:, sl].reshape(C, MB, P).transpose(2, 1, 0)).astype(np.float32)

    per_core = []
    for k in range(NCORES):
        sl = slice(k * S, (k + 1) * S)
        per_core.append({
            "b_sh": np.ascontiguousarray(B[:, :, :, sl]).astype(ml_dtypes.bfloat16),
            "g": g.astype(ml_dtypes.bfloat16),
            "r1p": _perp(r1, sl),
            "d2p": _perp(d2inv, sl),
            "lin1w": np.asarray(lin1_w, np.float32).astype(ml_dtypes.bfloat16),
            "lin1b": np.zeros((W_OUT, 1), np.float32),
            "lin2w": np.asarray(lin2_w, np.float32).astype(ml_dtypes.bfloat16),
        })
    return per_core


def timing_in_maps(inputs):
    return _host_prep(
        inputs["A"], inputs["h"], inputs["gt_w1a"], inputs["gt_w1b"],
        inputs["gt_w2"], inputs["gcn_w"], inputs["gcn_b"], inputs["lin1_w"],
        inputs["lin2_w"])


def build_timing(inputs, reps=1, nocc=False, stages=3, skipcc=False, warm=0):
    return _build(reps=reps, nocc=nocc, stages=stages, skipcc=skipcc, warm=warm)


def assemble(results, lin2_b):
    y = np.empty((N, NUM_CLASS), dtype=np.float32)
    for k in range(NCORES):
        y[k * S:(k + 1) * S, :] = results[k]["y_t"].T
    y += np.asarray(lin2_b, dtype=np.float32)[None, :]
    return y


def kernel(A, h, gt_w1a, gt_w1b, gt_w2, gcn_w, gcn_b, lin1_w, lin1_b, lin2_w,
           lin2_b, _run_kwargs=None):
    in_maps = _host_prep(A, h, gt_w1a, gt_w1b, gt_w2, gcn_w, gcn_b,
                         lin1_w, lin2_w)
    lb1 = np.asarray(lin1_b, np.float32).reshape(W_OUT, 1)
    for m in in_maps:
        m["lin1b"] = lb1

    nc = _build()
    res = run_bass_kernel_spmd(nc, in_maps, list(range(NCORES)),
                               **(_run_kwargs or {}))
    y = assemble(res.results, lin2_b)
    if _run_kwargs:
        kernel.last_results = res
    return y


# revision 18
# speedup vs baseline: 1.2291x; 1.2291x over previous
"""GTN Bass kernel v3 for 8 Trainium2 NeuronCores.

All three A-contractions run in the "A-stationary" orientation: the host
pre-combines the five edge-type adjacencies into per-channel, per-pass
matrices B[t,c] = sum_e f_t[c,e] A_e (softmax coefficients folded in), and
each pass computes X_out[n,w] = sum_m B[m,n] X_in[m,w] as 128x128-stationary
matmuls with the skinny [128,64] operand moving.  Per pass that is
C*MB*J = 64 matmuls of 64 moving columns (4096 PE column-cycles) versus the
v2 design's 80 matmuls of 256 columns (20480) - and no per-(c,e) DVE
prescales, because the coefficients live in B.

Pass outputs land node-major in PSUM ([node,64] per channel/block), which is
exactly the row layout the inter-pass AllGather wants, so the only
transposes left are the two [128,128] flips in front of the MLP tail (which
contracts over features).

Normalization vectors are the same exact host-side precomputes as v2:
  r1[c]  = 1/colsum(H1_c)   (pass-2 output row scaling)
  d2inv  = 1/(N*colsum(Q_c)) (pass-3 output scaling, GCN degree norm folded)
applied as per-partition scalars in the node-major layout.

Sharding: core k owns columns [256k, 256k+256) of every B[t,c] (host-sliced,
cast to bf16); AllGathers rebuild the full skinny operand between passes.
"""

import sys

import numpy as np

sys.path.insert(0, "/opt/trn_rl_repo")

import ml_dtypes

from concourse import bacc
import concourse.mybir as mybir
from concourse.bass import ds
from concourse.bass_utils import run_bass_kernel_spmd
from concourse.masks import make_identity
from concourse.tile import TileContext

E, C, N = 5, 2, 2048
W_IN, W_OUT, NUM_CLASS = 256, 64, 8
NCORES = 8
S = N // NCORES          # 256 shard columns per core
P = 128
J = N // P               # 16 contraction chunks
MB = S // P              # 2 output row blocks per shard
W2 = C * W_OUT           # 128: stacked channel width
T = 3                    # passes

F32 = mybir.dt.float32
BF16 = mybir.dt.bfloat16
ADD = mybir.AluOpType.add
MULT = mybir.AluOpType.mult
MAX = mybir.AluOpType.max
COPY = mybir.ActivationFunctionType.Copy


def _softmax(w):
    e = np.exp(w - w.max(axis=1, keepdims=True))
    return e / e.sum(axis=1, keepdims=True)


def _build(reps=1, nocc=False, stages=3, skipcc=False, warm=0):
    """Per-core SPMD program.  All softmax coefficients are folded into the
    host-combined B matrices; the program itself has no data-dependent
    immediates.  stages<3 truncates the per-rep body (timing experiments).
    skipcc replaces each AllGather with one local 64KB DMA (timing lower
    bound).  warm>0 emits that many dependency-free 64-col PE matmuls into
    each collective window so the PE p-state never drops."""
    nc = bacc.Bacc(None, target_bir_lowering=False)

    b_in = nc.declare_dram_parameter("b_sh", [T, C, N, S], BF16, isOutput=False)
    g_in = nc.declare_dram_parameter("g", [N, W_OUT], BF16, isOutput=False)
    r1_in = nc.declare_dram_parameter("r1p", [P, MB, C], F32, isOutput=False)
    d2_in = nc.declare_dram_parameter("d2p", [P, MB, C], F32, isOutput=False)
    l1_in = nc.declare_dram_parameter("lin1w", [W2, W_OUT], BF16, isOutput=False)
    b1_in = nc.declare_dram_parameter("lin1b", [W_OUT, 1], F32, isOutput=False)
    l2_in = nc.declare_dram_parameter("lin2w", [W_OUT, NUM_CLASS], BF16, isOutput=False)
    y_out = nc.declare_dram_parameter("y_t", [NUM_CLASS, S], F32, isOutput=True)

    ag1_in = nc.dram_tensor("ag1_in", [S, W2], BF16)
    ag1_out = nc.dram_tensor("ag1_out", [N, W2], BF16, addr_space="Shared")
    ag2_in = nc.dram_tensor("ag2_in", [S, W2], BF16)
    ag2_out = nc.dram_tensor("ag2_out", [N, W2], BF16, addr_space="Shared")
    groups = [list(range(NCORES))]

    with TileContext(nc) as tc:
        with (
            tc.tile_pool(name="bbuf", bufs=1) as b_pool,
            tc.tile_pool(name="wbuf", bufs=1) as w_pool,
            tc.tile_pool(name="mv", bufs=2) as mv_pool,
            tc.tile_pool(name="work", bufs=2) as wk,
            tc.tile_pool(name="psmain", bufs=1, space="PSUM") as pp,
            tc.tile_pool(name="pstr", bufs=1, space="PSUM") as pt,
            tc.tile_pool(name="pstail", bufs=1, space="PSUM") as pz,
        ):
            # ---- persistent SBUF loads -------------------------------------
            # row r of every row-indexed [N, *] operand maps to
            # (partition, chunk) = (r // J, r % J): per-partition DRAM reads
            # are fully contiguous.
            b_t = b_pool.tile([P, T, C, J, S], BF16, tag="B")
            nc.sync.dma_start(out=b_t[:, :, :, :, :],
                              in_=b_in[:].rearrange("t c (p j) m -> p t c j m",
                                                    p=P))
            g_t = w_pool.tile([P, J, W_OUT], BF16, tag="g")
            nc.sync.dma_start(out=g_t[:, :, :],
                              in_=g_in[:].rearrange("(p j) m -> p j m", p=P))
            r1_t = w_pool.tile([P, MB, C], F32, tag="r1")
            nc.sync.dma_start(out=r1_t[:, :, :], in_=r1_in[:])
            d2_t = w_pool.tile([P, MB, C], F32, tag="d2")
            nc.sync.dma_start(out=d2_t[:, :, :], in_=d2_in[:])
            l1_t = w_pool.tile([W2, W_OUT], BF16, tag="l1")
            nc.sync.dma_start(out=l1_t[:, :], in_=l1_in[:])
            b1_t = w_pool.tile([W_OUT, 1], F32, tag="b1")
            nc.sync.dma_start(out=b1_t[:, :], in_=b1_in[:])
            l2_t = w_pool.tile([W_OUT, NUM_CLASS], BF16, tag="l2")
            nc.sync.dma_start(out=l2_t[:, :], in_=l2_in[:])
            ident = w_pool.tile([P, P], BF16, tag="ident")
            make_identity(nc, ident[:, :])

            def run_pass(t, rhs_of, name, sink):
                """4 accumulation chains, nb-major, each [128,64] node-major.
                rhs_of(j, c) supplies the moving operand for chunk j; sink(c,
                nb, ps) consumes each chain's PSUM as it completes so the
                first half of the output can stage while the second half is
                still accumulating."""
                for nb in range(MB):
                    for c in range(C):
                        p_ = pp.tile([P, W_OUT], F32, tag=f"ps{c}{nb}",
                                     name=f"ps_{name}_{c}{nb}")
                        for j in range(J):
                            nc.tensor.matmul(
                                out=p_[:, :],
                                lhsT=b_t[:, t, c, j, ds(nb * P, P)],
                                rhs=rhs_of(j, c),
                                start=(j == 0),
                                stop=(j == J - 1),
                            )
                        sink(c, nb, p_)

            def run_warm(n, name):
                if n <= 0:
                    return
                psw = pp.tile([P, W_OUT], F32, tag="warm", name=f"warm_{name}")
                for i in range(n):
                    nc.tensor.matmul(out=psw[:, :],
                                     lhsT=b_t[:, 0, 0, 0, ds(0, P)],
                                     rhs=b_t[:, 0, 0, 1, ds(0, W_OUT)],
                                     start=True, stop=True)

            H = J // 2

            def all_gather(ag_in, ag_out, osb, name):
                nc.sync.dma_start(out=ag_in[:].rearrange("(m p) w -> p m w", p=P),
                                  in_=osb[:, :, :])
                if skipcc:
                    nc.sync.dma_start(out=ag_out[ds(0, S), :], in_=ag_in[:])
                elif nocc:
                    for kk in range(NCORES):
                        nc.sync.dma_start(out=ag_out[ds(kk * S, S), :],
                                          in_=ag_in[:])
                else:
                    nc.gpsimd.collective_compute(
                        "AllGather", mybir.AluOpType.bypass,
                        replica_groups=groups,
                        ins=[ag_in[:]], outs=[ag_out[:]])
                run_warm(warm, name)
                mv = mv_pool.tile([P, J, W2], BF16, tag="mvin", name=f"mv_{name}")
                nc.sync.dma_start(out=mv[:, :, :],
                                  in_=ag_out[:].rearrange("(p j) m -> p j m", p=P))
                return lambda j, c: mv[:, j, ds(W_OUT * c, W_OUT)]

            prev_tail = [None]
            for _rep in range(reps):
                if _rep > 0 and prev_tail[0] is not None:
                    # serialize reps so the reps-slope measures latency
                    zt = wk.tile([NUM_CLASS, 1], F32, tag="zdep",
                                 name=f"zdep_{_rep}")
                    nc.vector.tensor_scalar(zt[:, :], prev_tail[0],
                                            0.0, None, MULT)
                    nc.vector.tensor_tensor(g_t[0:NUM_CLASS, 0, ds(0, 1)],
                                            g_t[0:NUM_CLASS, 0, ds(0, 1)],
                                            zt[:, :], ADD)

                # ---- pass 1: X1[c] = B[0,c]^T g ----------------------------
                osb1 = wk.tile([P, MB, W2], BF16, tag="osb1", name=f"osb1_{_rep}")
                run_pass(0, lambda j, c: g_t[:, j, :], f"p1_{_rep}",
                         lambda c, nb, p_: nc.vector.tensor_copy(
                             osb1[:, nb, ds(W_OUT * c, W_OUT)], p_[:, :]))
                if stages < 2:
                    prev_tail[0] = osb1[0:NUM_CLASS, 0, ds(0, 1)]
                    continue
                mv1 = all_gather(ag1_in, ag1_out, osb1, f"1_{_rep}")

                # ---- pass 2: X2[c] = r1[c] * (B[1,c]^T X1[c]) --------------
                osb2 = wk.tile([P, MB, W2], BF16, tag="osb2", name=f"osb2_{_rep}")
                run_pass(1, mv1, f"p2_{_rep}",
                         lambda c, nb, p_: nc.vector.tensor_scalar(
                             osb2[:, nb, ds(W_OUT * c, W_OUT)], p_[:, :],
                             r1_t[:, nb, ds(c, 1)], None, MULT))
                if stages < 3:
                    prev_tail[0] = osb2[0:NUM_CLASS, 0, ds(0, 1)]
                    continue
                mv2 = all_gather(ag2_in, ag2_out, osb2, f"2_{_rep}")

                # ---- pass 3 + GCN scale/relu + transpose + MLP tail --------
                xt = wk.tile([P, MB, W2], BF16, tag="xt", name=f"xt_{_rep}")
                run_pass(2, mv2, f"p3_{_rep}",
                         lambda c, nb, p_: nc.vector.tensor_scalar(
                             xt[:, nb, ds(W_OUT * c, W_OUT)], p_[:, :],
                             d2_t[:, nb, ds(c, 1)], 0.0, MULT, MAX))
                xb = wk.tile([P, S], BF16, tag="xb", name=f"xb_{_rep}")
                for nb in range(MB):
                    pst = pt.tile([P, P], BF16, tag="pst", name=f"t3_{_rep}_{nb}")
                    nc.tensor.transpose(pst[:, :], xt[:, nb, :], ident[:, :])
                    nc.vector.tensor_copy(xb[:, ds(nb * P, P)], pst[:, :])
                psz = pz.tile([W_OUT, S], F32, tag="psz", name=f"psz_{_rep}")
                nc.tensor.matmul(out=psz[:, :], lhsT=l1_t[:, :], rhs=xb[:, :],
                                 start=True, stop=True)
                z = wk.tile([W_OUT, S], BF16, tag="z", name=f"z_{_rep}")
                nc.vector.tensor_scalar(z[:, :], psz[:, :], b1_t[:, ds(0, 1)],
                                        0.0, ADD, MAX)
                psy = pz.tile([NUM_CLASS, S], F32, tag="psy", name=f"psy_{_rep}")
                nc.tensor.matmul(out=psy[:, :], lhsT=l2_t[:, :], rhs=z[:, :],
                                 start=True, stop=True)
                ysb = wk.tile([NUM_CLASS, S], F32, tag="ysb", name=f"ysb_{_rep}")
                nc.vector.tensor_copy(ysb[:, :], psy[:, :])
                nc.sync.dma_start(out=y_out[:, :], in_=ysb[:, :])
                run_warm(warm // 2, f"t_{_rep}")
                prev_tail[0] = ysb[:, ds(0, 1)]

    nc.finalize()
    return nc


def _host_prep(A, h, gt_w1a, gt_w1b, gt_w2, gcn_w, gcn_b, lin1_w, lin2_w):
    A = np.asarray(A, dtype=np.float32)
    f1a = _softmax(np.asarray(gt_w1a, dtype=np.float64))
    f1b = _softmax(np.asarray(gt_w1b, dtype=np.float64))
    f2 = _softmax(np.asarray(gt_w2, dtype=np.float64))

    g = (np.asarray(h, np.float32) @ np.asarray(gcn_w, np.float32)
         + np.asarray(gcn_b, np.float32))                       # [N, 64]

    # B[t,c] = sum_e f_t[c,e] A_e, per pass t in (f1a, f1b, f2)
    fs = np.stack([f1a, f1b, f2]).astype(np.float32)            # [T, C, E]
    B = np.einsum('tce,enm->tcnm', fs, A)                       # [T, C, N, N]

    Ad = A.astype(np.float64)
    sA = Ad.sum(axis=1)                                         # [E, N] colsums
    s1 = f1a @ sA                                               # [C, N]
    # u1[c] = colsum(H1_c) = sum_e f1b[c,e] * (A_e.T @ s1[c])
    u1 = np.stack([
        sum(f1b[c, e] * (Ad[e].T @ s1[c]) for e in range(E)) for c in range(C)
    ])                                                          # [C, N]
    r1 = np.where(u1 != 0, 1.0 / u1, 0.0)
    deg2 = f2 @ sA                                              # [C, N]
    d2inv = np.where(deg2 != 0, 1.0 / (N * deg2), 0.0)          # [C, N]

    def _perp(v, sl):
        # [C, S] slice -> [P, MB, C] per-partition scalars (node-major)
        return np.ascontiguousarray(
            v[:, sl].reshape(C, MB, P).transpose(2, 1, 0)).astype(np.float32)

    per_core = []
    for k in range(NCORES):
        sl = slice(k * S, (k + 1) * S)
        per_core.append({
            "b_sh": np.ascontiguousarray(B[:, :, :, sl]).astype(ml_dtypes.bfloat16),
            "g": g.astype(ml_dtypes.bfloat16),
            "r1p": _perp(r1, sl),
            "d2p": _perp(d2inv, sl),
            "lin1w": np.asarray(lin1_w, np.float32).astype(ml_dtypes.bfloat16),
            "lin1b": np.zeros((W_OUT, 1), np.float32),
            "lin2w": np.asarray(lin2_w, np.float32).astype(ml_dtypes.bfloat16),
        })
    return per_core


def timing_in_maps(inputs):
    return _host_prep(
        inputs["A"], inputs["h"], inputs["gt_w1a"], inputs["gt_w1b"],
        inputs["gt_w2"], inputs["gcn_w"], inputs["gcn_b"], inputs["lin1_w"],
        inputs["lin2_w"])


def build_timing(inputs, reps=1, nocc=False, stages=3, skipcc=False, warm=0):
    return _build(reps=reps, nocc=nocc, stages=stages, skipcc=skipcc, warm=warm)


def assemble(results, lin2_b):
    y = np.empty((N, NUM_CLASS), dtype=np.float32)
    for k in range(NCORES):
        y[k * S:(k + 1) * S, :] = results[k]["y_t"].T
    y += np.asarray(lin2_b, dtype=np.float32)[None, :]
    return y


def kernel(A, h, gt_w1a, gt_w1b, gt_w2, gcn_w, gcn_b, lin1_w, lin1_b, lin2_w,
           lin2_b, _run_kwargs=None):
    in_maps = _host_prep(A, h, gt_w1a, gt_w1b, gt_w2, gcn_w, gcn_b,
                         lin1_w, lin2_w)
    lb1 = np.asarray(lin1_b, np.float32).reshape(W_OUT, 1)
    for m in in_maps:
        m["lin1b"] = lb1

    nc = _build()
    res = run_bass_kernel_spmd(nc, in_maps, list(range(NCORES)),
                               **(_run_kwargs or {}))
    y = assemble(res.results, lin2_b)
    if _run_kwargs:
        kernel.last_results = res
    return y
